# revision 21
# baseline (speedup 1.0000x reference)
"""Trainium2 Bass kernel for nn_CAD_GCN (gnn_message_passing).

Math: with x [B,C,H,W], S = H*W, x_node = mean_s x,
  h   = x_node @ g1_w.T + g1_b
  z1  = h*g2_w + g2_b
  y   = sum_n (theta_w x + theta_b)[n] * z1[n]
      = sum_c w_eff[c]*x[c,s] + bias_eff          (no Bmap materialization)
  out = tanh(x + phi_w[c]*y + phi_b[c])
where w_eff = x_node @ A + r, bias_eff = x_node @ a + s0 with
  A = g2_w*(g1_w.T @ theta_w), r = (g2_w*g1_b + g2_b) @ theta_w
  a = g2_w*(g1_w.T @ theta_b), s0 = (g2_w*g1_b + g2_b) @ theta_b
(all host-precomputable from the tiny parameter tensors).

Sharding: pure data parallel, 2 samples per core on 8 cores.

The error gate (2e-2 absmax-relative) admits bf16 at the HBM boundary:
x is uploaded as bf16 and the output is downloaded as bf16 (host casts
either way; worst-case output perturbation ~5e-3).  That halves DMA
traffic AND lets loads land directly in the retained SBUF image of x —
no staging, no convert pass.  Per core, per sample (viewed as
[128, 32768] bf16, partition p = (c, half)):

  loads:  16x 2048-col DMAs straight into the resident xret tile.
  sums:   DVE in-place tensor_scalar(mult 1.0) per chunk with fused
          accum_out — runs in the DVE 16-bit fast mode (689ns/2048),
          comfortably ahead of the 1456ns/chunk DMA stream.
  tiny:   w_eff/bias via two small PE matmuls + DVE;
          M2b = I + parity*(w_eff outer phi) in bf16, so pass 2 is
          z = M2b.T @ x = x + phi*y directly (no elementwise add).
  pass 2: per 2048-col PSUM tile (4 banks, 2-deep ring): 4 bf16 PE
          matmuls + ONE 2048-wide ACT tanh(bias) into bf16 staging
          (10-deep ring so ACT never stalls), stored per 2048 cols
          from gpsimd.

DMA busy is ~94us (46.6 in + 46.6 out + consts) and runs gapless; the
ACT tanh stream (32x 1882ns = 60us) plus sample A's load phase fits
just under it.  TimelineSim: 97.7us = 2.0 head + 93.9 DMA + 1.8 tail
(vs 233.4us baseline).
"""

import sys

for _p in ("/opt/trn_rl_repo",):
    if _p not in sys.path:
        sys.path.insert(0, _p)

import numpy as np

import concourse.bacc as bacc
import concourse.bass as bass
import concourse.mybir as mybir
import concourse.tile as tile
from concourse.bass_utils import run_bass_kernel_spmd

F32 = mybir.dt.float32
BF16 = mybir.dt.bfloat16
NP_BF16 = mybir.dt.np(BF16)

B, C, H, W = 16, 64, 256, 256
S = H * W                      # 65536 pixels per sample
NCORES = 8
BPC = B // NCORES              # 2 samples per core
P = 128                        # SBUF partitions; per sample p = 2*c + half
SPS = S // 2                   # 32768 pixels per virtual half-sample column
INV_S = 1.0 / float(S)

LC = 2048                      # load chunk (bf16 cols)
SC = 2048                      # store chunk (bf16 cols)
ZW = 2048                      # PSUM z tile width (4 banks)
NLOAD = SPS // LC              # 16 loads per sample
NZ = SPS // ZW                 # 16 z-chunks per sample
ZPS = SC // ZW                 # z-chunks per store (2)


def _build_program(ot_bufs=10, psz_bufs=2, load_eng="sync", store_eng="gpsimd"):
    nc = bacc.Bacc("TRN2", target_bir_lowering=False, debug=False)

    x_d = nc.dram_tensor("x", [2 * P, SPS], BF16, kind="ExternalInput")
    consts_d = nc.dram_tensor("consts", [P, 516], F32, kind="ExternalInput")
    out_d = nc.dram_tensor("out", [2 * P, SPS], BF16, kind="ExternalOutput")

    X = mybir.AxisListType.X
    Tanh = mybir.ActivationFunctionType.Tanh
    Mult = mybir.AluOpType.mult
    Add = mybir.AluOpType.add

    with tile.TileContext(nc) as tc:
        with (
            tc.tile_pool(name="consts", bufs=1) as cpool,
            tc.tile_pool(name="xret", bufs=1) as rpool,
            tc.tile_pool(name="stats", bufs=1) as stats,
            tc.tile_pool(name="opool", bufs=ot_bufs) as opool,
            tc.tile_pool(name="ps_z", bufs=psz_bufs, space="PSUM") as ps_z,
        ):
            consts_sb = cpool.tile([P, 516], F32, name="consts_sb")
            getattr(nc, load_eng).dma_start(consts_sb[:], consts_d[:])
            mbd_sb = consts_sb[:, 0:128]
            abd_sb = consts_sb[:, 128:256]
            pphi_sb = consts_sb[:, 256:384]
            eye_sb = consts_sb[:, 384:512]
            rcol_sb = consts_sb[:, 512:513]
            bcol_sb = consts_sb[:, 513:514]

            xret = [rpool.tile([P, SPS], BF16, name=f"xret{s}") for s in range(2)]
            snk = [stats.tile([P, NLOAD], F32, name=f"snk{s}") for s in range(2)]

            def emit_load_sum(s, i):
                sl = slice(i * LC, (i + 1) * LC)
                getattr(nc, load_eng).dma_start(
                    xret[s][:, sl], x_d[s * P : (s + 1) * P, sl]
                )
                # in-place mult-by-1 (bitwise identity on bf16) purely to
                # ride the DVE 16-bit fast path while harvesting the sum
                with nc.allow_low_precision(reason="bf16 identity copy; sum accumulates in f32"):
                    nc.vector.tensor_scalar(
                        xret[s][:, sl], xret[s][:, sl], 1.0, 0.0, Mult, Add,
                        accum_out=snk[s][:, i : i + 1],
                    )

            M2b = [None, None]
            bias2 = [None, None]

            def emit_tiny(s):
                sums = stats.tile([P, 1], F32, name=f"sums{s}")
                nc.vector.reduce_sum(sums[:, 0:1], snk[s][:], X)
                w2raw = ps_z.tile([P, ZW], F32, name="w2raw", tag="z")
                nc.tensor.matmul(w2raw[:, 0:1], mbd_sb, sums[:, 0:1], start=True, stop=True)
                b2raw = ps_z.tile([P, ZW], F32, name="b2raw", tag="z")
                nc.tensor.matmul(b2raw[:, 0:1], abd_sb, sums[:, 0:1], start=True, stop=True)
                w2col = stats.tile([P, 1], F32, name=f"w2col{s}")
                nc.vector.tensor_add(w2col[:], w2raw[:, 0:1], rcol_sb)
                b2 = stats.tile([P, 1], F32, name=f"bias2_{s}")
                nc.vector.tensor_add(b2[:], b2raw[:, 0:1], bcol_sb)
                m2t = stats.tile([P, P], F32, name=f"m2t{s}")
                nc.vector.tensor_scalar_mul(m2t[:], pphi_sb, w2col[:, 0:1])
                m2b = stats.tile([P, P], BF16, name=f"m2b{s}")
                nc.vector.tensor_add(m2b[:], m2t[:], eye_sb)
                M2b[s] = m2b
                bias2[s] = b2

            ot_cur = [None, None]

            def emit_z_chunk(s, zi):
                if zi % ZPS == 0:
                    ot_cur[s] = opool.tile([P, SC], BF16, name="ot", tag="ot")
                ot = ot_cur[s]
                z_ps = ps_z.tile([P, ZW], F32, name="z", tag="z")
                for j in range(ZW // 512):
                    g0 = zi * ZW + j * 512
                    nc.tensor.matmul(
                        z_ps[:, j * 512 : (j + 1) * 512], M2b[s][:],
                        xret[s][:, g0 : g0 + 512], start=True, stop=True,
                    )
                off = (zi % ZPS) * ZW
                nc.scalar.activation(
                    ot[:, off : off + ZW], z_ps[:], Tanh, bias=bias2[s][:, 0:1]
                )
                if zi % ZPS == ZPS - 1:
                    si = zi // ZPS
                    getattr(nc, store_eng).dma_start(
                        out_d[s * P : (s + 1) * P, si * SC : (si + 1) * SC], ot[:]
                    )

            # phase 1: sample A in
            for i in range(NLOAD):
                emit_load_sum(0, i)
            emit_tiny(0)
            # phase 2: sample B in, 1:1 with A pass 2
            za = 0
            for i in range(NLOAD):
                emit_load_sum(1, i)
                if za < NZ:
                    emit_z_chunk(0, za)
                    za += 1
            emit_tiny(1)
            # phase 3: finish A (nothing left when NLOAD==NZ), then B
            while za < NZ:
                emit_z_chunk(0, za)
                za += 1
            for zb in range(NZ):
                emit_z_chunk(1, zb)

    nc.compile()
    return nc


def _host_consts(theta_w, theta_b, g1_w, g1_b, g2_w, g2_b, phi_w, phi_b):
    """Fold the GCN parameter chain into one packed [128, 516] tensor."""
    f8 = np.float64
    theta_w = theta_w.astype(f8)
    theta_b = theta_b.astype(f8)
    g1_w = g1_w.astype(f8)
    g1_b = g1_b.astype(f8)
    g2w = f8(g2_w.reshape(-1)[0])
    g2b = f8(g2_b.reshape(-1)[0])
    phi_w = phi_w.astype(f8)
    phi_b = phi_b.astype(f8)

    A = g2w * (g1_w.T @ theta_w)            # [C, C]
    r = (g2w * g1_b + g2b) @ theta_w        # [C]
    a = g2w * (g1_w.T @ theta_b)            # [C]
    s0 = (g2w * g1_b + g2b) @ theta_b       # scalar

    rep = lambda v: np.repeat(v, 2)         # c = p // 2
    # w2col[p'] = sum_p mbd[p,p'] * sums[p] (+ rcol) = w_eff[c(p')]
    mbd = np.repeat(np.repeat(A, 2, axis=0), 2, axis=1) * INV_S
    # b2[p'] = sum_p abd[p,p'] * sums[p] (+ bcol) = phi_w[c(p')]*s_b + ...
    abd = np.outer(rep(a), rep(phi_w)) * INV_S
    # pphi[p,p'] = (p%2 == p'%2) * phi_w[c(p')]
    par = (np.arange(P)[:, None] % 2) == (np.arange(P)[None, :] % 2)
    pphi = par * rep(phi_w)[None, :]
    rcol = rep(r)
    bcol = rep(phi_w * s0 + phi_b)

    consts = np.zeros((P, 516), f8)
    consts[:, 0:128] = mbd
    consts[:, 128:256] = abd
    consts[:, 256:384] = pphi
    consts[:, 384:512] = np.eye(P)
    consts[:, 512] = rcol
    consts[:, 513] = bcol
    return np.ascontiguousarray(consts, dtype=np.float32)


_NC_CACHE = {}


def _get_nc():
    key = (S, LC, SC, ZW)
    if key not in _NC_CACHE:
        _NC_CACHE[key] = _build_program()
    return _NC_CACHE[key]


def _run(inputs, trace=False):
    x = np.asarray(inputs["x"]).astype(NP_BF16)
    consts = _host_consts(
        np.asarray(inputs["theta_w"]), np.asarray(inputs["theta_b"]),
        np.asarray(inputs["g1_w"]), np.asarray(inputs["g1_b"]),
        np.asarray(inputs["g2_w"]), np.asarray(inputs["g2_b"]),
        np.asarray(inputs["phi_w"]), np.asarray(inputs["phi_b"]),
    )
    in_maps = []
    for k in range(NCORES):
        xk = x[k * BPC : (k + 1) * BPC].reshape(2 * P, SPS)
        in_maps.append({"x": np.ascontiguousarray(xk), "consts": consts})

    nc = _get_nc()
    res = run_bass_kernel_spmd(
        nc, in_maps, core_ids=list(range(NCORES)), trace=trace
    )
    out = np.empty((B, C, H, W), dtype=np.float32)
    for k in range(NCORES):
        out[k * BPC : (k + 1) * BPC] = (
            np.asarray(res.results[k]["out"])
            .astype(np.float32)
            .reshape(BPC, C, H, W)
        )
    return out, res


def kernel(**inputs):
    out, _ = _run(inputs, trace=False)
    return out
